# revision 16
# baseline (speedup 1.0000x reference)
"""Trainium2 kernel for nn_BlockLinear: gather -> per-block GEMM -> scatter-add.

The whole op is linear in x, so gather/einsum/scatter fold into one dense GEMM
out[t, o] = sum_k x[t, k] * Wfull[k, o] + bias[o], with Wfull built on host
(bincount scatter-add, exact fp64 accumulation). The GEMM runs on 8
NeuronCores, sharded 2D: 4 token groups x 2 out-feature groups.

Mixed-precision contraction split (rel-err budget 2e-2, measured 1.8e-2):
the first N8*256 of K runs as fp8(e4m3) DoubleRow matmuls (256-contraction per
instruction, 2x PE throughput), the remaining K in fp16. Operands are
pre-scaled by powers of two (x*2^5, w*2^10) so both parts accumulate in one
fp32 PSUM group; drains rescale by 2^-15 and add the bias in one fused op.

Schedule: diagonally-staggered warmup over 4 o-groups (k-major, consuming the
x stream as it arrives), then o-major steady phase with chain boundaries
software-pipelined (the next o-group's first matmul issues before the current
group's last, hiding the chain-start LDWEIGHTS hiccup).
"""

import numpy as np
import ml_dtypes
import concourse.bacc as bacc
import concourse.mybir as mybir
import concourse.tile as tile
from concourse.bass_utils import run_bass_kernel_spmd

# problem shapes (hardcoded per contract)
B, S = 2, 2048
IN_FEATURES = 4096
OUT_FEATURES = 4096
NTOKENS = B * S                  # 4096

NCORES = 8
TG, OG = 4, 2                    # token groups x out-feature groups
T = NTOKENS // TG                # 1024 tokens per core
O = OUT_FEATURES // OG           # 2048 out features per core
P = 128
KT = IN_FEATURES // P            # 32 contraction tiles
OT = O // P                      # 16 out-feature tiles per core
NTOK = 512                       # moving free dim per matmul
TB = T // NTOK                   # 2 token blocks per core

N8 = 7                           # fp8 DoubleRow pairs (256 K each)
K8 = N8 * 2 * P                  # K prefix contracted in fp8
KB = KT - N8 * 2                 # fp16 k-tiles

SX = 32.0                        # x pre-scale (power of 2)
SW = 1024.0                      # w pre-scale (power of 2)
SCALE_OUT = 1.0 / (SX * SW)      # exact 2^-15

F32 = mybir.dt.float32
HP = mybir.dt.float16            # dtype of the non-fp8 contraction part
FP8 = mybir.dt.float8e4
DR = mybir.MatmulPerfMode.DoubleRow
NP_FP8 = ml_dtypes.float8_e4m3   # TRN FP8_EXP4: max normal 240
NP_HP = np.float16

# knobs for test.py
TRACE = False
LAST_RESULTS = None

# contraction units per (o, tb) psum chain: N8 DoubleRow + KB fp16
UNITS = [("dr", i) for i in range(N8)] + [("bf", i) for i in range(KB)]
NU = len(UNITS)                  # 26
# warmup consumption order: DR units interleaved into the fp16 stream so the
# later x8 slabs (which arrive behind wb on the scalar queue) are needed later
WUNITS = [0, 1, 7, 8, 2, 9, 10, 3, 11, 12, 4, 13, 14, 5, 15, 16, 6] + list(
    range(17, 25)
)
assert sorted(WUNITS) == list(range(NU))
LASTW = 4                        # trailing units drained o-major in warmup
NWARM = 4                        # o-groups processed k-major during warmup
STAG = 2                         # warmup diagonal stagger (units per o-group)


def build_nc():
    nc = bacc.Bacc()
    x8 = nc.dram_tensor("x8", [N8, P, 2, T], FP8, kind="ExternalInput")
    xb = nc.dram_tensor("xb", [KB, P, T], HP, kind="ExternalInput")
    w8 = nc.dram_tensor("w8", [OT, P, N8, 2, P], FP8, kind="ExternalInput")
    wb = nc.dram_tensor("wb", [OT, P, KB, P], HP, kind="ExternalInput")
    bo = nc.dram_tensor("bo", [P, OT], F32, kind="ExternalInput")
    out = nc.dram_tensor("out", [OT, TB, P, NTOK], F32, kind="ExternalOutput")

    with tile.TileContext(nc) as tc:
        with (
            tc.tile_pool(name="x_sb", bufs=1) as x_sb,
            tc.tile_pool(name="w8_sb", bufs=7) as w8_sb,
            tc.tile_pool(name="wb_sb", bufs=6) as wb_sb,
            tc.tile_pool(name="o_sb", bufs=4) as o_sb,
            tc.tile_pool(name="ps", bufs=8, space="PSUM") as ps,
        ):
            bo_t = x_sb.tile([P, OT], F32, tag="bo")

            # PE HAM warmup: dummy matmuls on memset data fill the dead time
            # while the first DMAs land, so real matmuls start at 2.4 GHz
            dummy_sb = x_sb.tile([P, NTOK], HP, tag="dummy")
            nc.vector.memset(dummy_sb.bitcast(F32), 0.0)
            ps_d = ps.tile([P, NTOK], F32, tag="ps", name="ps_dummy")
            for _ in range(10):
                nc.tensor.matmul(
                    ps_d, dummy_sb[:, :P], dummy_sb, start=True, stop=True
                )

            w8t, wbt, x8_t, xb_t = {}, {}, {}, {}

            def load_w8(o):
                t = w8_sb.tile([P, N8, 2, P], FP8, tag="w8t", name=f"w8_{o}")
                nc.sync.dma_start(out=t, in_=w8[o])
                w8t[o] = t

            def load_wb(o, eng):
                t = wb_sb.tile([P, KB, P], HP, tag="wbt", name=f"wb_{o}")
                eng.dma_start(out=t, in_=wb[o])
                wbt[o] = t

            def load_x8(i, eng):
                t = x_sb.tile([P, 2, T], FP8, tag=f"x8_{i}")
                eng.dma_start(out=t, in_=x8[i])
                x8_t[i] = t

            def load_xb(i, eng):
                t = x_sb.tile([P, T], HP, tag=f"xb_{i}")
                eng.dma_start(out=t, in_=xb[i])
                xb_t[i] = t

            def unit_mm(psum, o, u, tb, start, stop):
                kind, i = UNITS[u]
                if kind == "dr":
                    nc.tensor.matmul(
                        psum,
                        w8t[o][:, i],
                        x8_t[i][:, :, tb * NTOK : (tb + 1) * NTOK],
                        start=start,
                        stop=stop,
                        perf_mode=DR,
                    )
                else:
                    nc.tensor.matmul(
                        psum,
                        wbt[o][:, i],
                        xb_t[i][:, tb * NTOK : (tb + 1) * NTOK],
                        start=start,
                        stop=stop,
                    )

            o_pair = {}

            def drain(o, tb, psum):
                if tb == 0:
                    o_pair[o] = o_sb.tile([P, TB, NTOK], F32, tag="ot", name=f"ot_{o}")
                o_t = o_pair[o][:, tb]
                # psum * 2^-15 + bias in one fused op; alternate engines so
                # consecutive drains run in parallel
                if (o * TB + tb) % 2 == 0:
                    nc.scalar.activation(
                        o_t,
                        psum,
                        mybir.ActivationFunctionType.Identity,
                        bias=bo_t[:, o : o + 1],
                        scale=SCALE_OUT,
                    )
                else:
                    nc.vector.tensor_scalar(
                        o_t,
                        psum,
                        SCALE_OUT,
                        bo_t[:, o : o + 1],
                        mybir.AluOpType.mult,
                        mybir.AluOpType.add,
                    )
                if tb == TB - 1:
                    nc.scalar.dma_start(
                        out=out[o].rearrange("two p n -> p two n"), in_=o_pair[o]
                    )

            # ---- DMA issue for the warmup span ----
            # x8 + w8 go on the two hardware-DGE queues (sync/scalar) in
            # consumption order -- gpsimd's software DGE adds latency, so it
            # only carries the latency-tolerant xb bulk stream
            load_x8(0, nc.sync)
            load_w8(0)
            load_x8(1, nc.sync)
            for o in range(1, NWARM):
                load_w8(o)
            nc.sync.dma_start(out=bo_t, in_=bo[:, :])
            load_wb(0, nc.scalar)
            load_wb(1, nc.scalar)
            load_x8(2, nc.scalar)
            load_x8(3, nc.scalar)
            load_wb(2, nc.scalar)
            load_x8(4, nc.scalar)
            load_wb(3, nc.scalar)
            load_x8(5, nc.scalar)
            load_x8(6, nc.scalar)
            for i in range(13):
                load_xb(i, nc.gpsimd)
            for i in range(13, KB):
                load_xb(i, nc.scalar)

            # ---- warmup: diagonally staggered k-major over NWARM o-groups ----
            # o-group o starts STAG units late so its weights needn't arrive
            # all at once with o=0's
            psums = {
                (o, tb): ps.tile([P, NTOK], F32, tag="ps", name=f"psw_{o}_{tb}")
                for o in range(NWARM)
                for tb in range(TB)
            }
            NWU = NU - LASTW
            for s in range(NWU + STAG * (NWARM - 1)):
                for o in range(NWARM):
                    idx = s - STAG * o
                    if 0 <= idx < NWU:
                        for tb in range(TB):
                            unit_mm(psums[o, tb], o, WUNITS[idx], tb, idx == 0, False)

            # ---- unified tail: warmup chain closings + steady o-groups, with
            # chain boundaries software-pipelined (next group's first unit
            # issues before this group's last unit + drains)
            pend = None
            for o in range(OT):
                if o < NWARM:
                    psg = {tb: psums[o, tb] for tb in range(TB)}
                    order = WUNITS[NU - LASTW :]
                    opens = False
                else:
                    load_w8(o)
                    load_wb(o, nc.sync if o % 2 == 0 else nc.scalar)
                    psg = {
                        tb: ps.tile([P, NTOK], F32, tag="ps", name=f"ps_{o}_{tb}")
                        for tb in range(TB)
                    }
                    # alternate chain direction so fp8 LDWEIGHTS bursts don't
                    # pile up at every boundary
                    order = list(range(NU)) if o % 2 == 0 else list(range(NU))[::-1]
                    opens = True
                for tb in range(TB):
                    unit_mm(psg[tb], o, order[0], tb, opens, False)
                if pend is not None:
                    po, pps, plast = pend
                    for tb in range(TB):
                        unit_mm(pps[tb], po, plast, tb, False, True)
                    for tb in range(TB):
                        drain(po, tb, pps[tb])
                for u in order[1:-1]:
                    for tb in range(TB):
                        unit_mm(psg[tb], o, u, tb, False, False)
                pend = (o, psg, order[-1])
            po, pps, plast = pend
            for tb in range(TB):
                unit_mm(pps[tb], po, plast, tb, False, True)
            for tb in range(TB):
                drain(po, tb, pps[tb])
    nc.finalize()
    return nc


_NC = None


def _get_nc():
    global _NC
    if _NC is None:
        _NC = build_nc()
    return _NC


def _build_wfull(weights, input_indices, output_indices):
    """Wfull[k, o] = sum over blocks/dups of weights[n, j, i]."""
    ii = np.asarray(input_indices).astype(np.int64)     # [NBLK, BI]
    oi = np.asarray(output_indices).astype(np.int64)    # [NBLK, BO]
    w = np.asarray(weights, dtype=np.float64)           # [NBLK, BO, BI]
    flat = (ii[:, :, None] * OUT_FEATURES + oi[:, None, :]).ravel()  # [n, i, j]
    vals = np.ascontiguousarray(np.swapaxes(w, 1, 2)).ravel()        # [n, i, j]
    wfull = np.bincount(flat, weights=vals, minlength=IN_FEATURES * OUT_FEATURES)
    return wfull.reshape(IN_FEATURES, OUT_FEATURES)


def _to_fp8(a):
    return np.clip(a, -240.0, 240.0).astype(NP_FP8)


def prepare_in_maps(x, weights, bias, input_indices, output_indices):
    x = np.asarray(x, dtype=np.float32).reshape(NTOKENS, IN_FEATURES)
    bias = np.asarray(bias, dtype=np.float32)
    wfull = _build_wfull(weights, input_indices, output_indices)

    # permute k-tiles so the 2*N8 with the least quantization-error energy
    # (sum_k ||x[:,k]||^2 * ||w[k,:]||^2) are the ones computed in fp8
    xsq = (x.astype(np.float64) ** 2).sum(0)
    wsq = (wfull ** 2).sum(1)
    mass = (xsq * wsq).reshape(KT, P).sum(1)
    order = np.argsort(mass)
    perm = np.concatenate([np.sort(order[: 2 * N8]), np.sort(order[2 * N8 :])])
    kperm = (perm[:, None] * P + np.arange(P)).ravel()
    x = x[:, kperm]
    wfull = wfull[kperm, :]

    # quantize once globally (scales are powers of two; folded out in drain)
    x8_full = _to_fp8(x[:, :K8].astype(np.float64) * SX)         # [NT, K8]
    xb_full = (x[:, K8:] * np.float32(SX)).astype(NP_HP)         # [NT, K-K8]
    w8_full = _to_fp8(wfull[:K8, :] * SW)                        # [K8, OF]
    wb_full = (wfull[K8:, :] * SW).astype(NP_HP)                 # [K-K8, OF]

    in_maps = []
    for c in range(NCORES):
        tg, og = divmod(c, OG)
        tsl = slice(tg * T, (tg + 1) * T)
        osl = slice(og * O, (og + 1) * O)
        # x8: [N8, P, 2, T] ; k = (2*kk + j)*128 + p
        x8c = np.ascontiguousarray(
            x8_full[tsl].T.reshape(N8, 2, P, T).transpose(0, 2, 1, 3)
        )
        # xb: [KB, P, T]
        xbc = np.ascontiguousarray(xb_full[tsl].T.reshape(KB, P, T))
        # w8: [OT, P, N8, 2, P]
        w8c = np.ascontiguousarray(
            w8_full[:, osl].reshape(N8, 2, P, OT, P).transpose(3, 2, 0, 1, 4)
        )
        # wb: [OT, P, KB, P]
        wbc = np.ascontiguousarray(
            wb_full[:, osl].reshape(KB, P, OT, P).transpose(2, 1, 0, 3)
        )
        boc = np.ascontiguousarray(bias[osl].reshape(OT, P).T)
        in_maps.append({"x8": x8c, "xb": xbc, "w8": w8c, "wb": wbc, "bo": boc})
    return in_maps


def assemble_output(core_outs):
    full = np.empty((NTOKENS, OUT_FEATURES), np.float32)
    for c in range(NCORES):
        tg, og = divmod(c, OG)
        o4 = np.asarray(core_outs[c])                    # [OT, TB, P, NTOK]
        blk = o4.transpose(1, 3, 0, 2).reshape(T, O)     # [t, o]
        full[tg * T : (tg + 1) * T, og * O : (og + 1) * O] = blk
    return full.reshape(B, S, OUT_FEATURES)


def kernel(x, weights, bias, input_indices, output_indices):
    global LAST_RESULTS
    in_maps = prepare_in_maps(x, weights, bias, input_indices, output_indices)
    nc = _get_nc()
    res = run_bass_kernel_spmd(nc, in_maps, list(range(NCORES)))
    LAST_RESULTS = res
    return assemble_output([res.results[c]["out"] for c in range(NCORES)])


# revision 18
# speedup vs baseline: 1.1492x; 1.1492x over previous
"""Trainium2 kernel for nn_BlockLinear: gather -> per-block GEMM -> scatter-add.

The whole op is linear in x, so gather/einsum/scatter fold into one dense GEMM
out[t, o] = sum_k x[t, k] * Wfull[k, o] + bias[o], with Wfull built on host
(bincount scatter-add, exact fp64 accumulation). The GEMM runs on 8
NeuronCores, sharded 2D: 4 token groups x 2 out-feature groups.

Mixed-precision contraction split (rel-err budget 2e-2, measured 1.8e-2):
the first N8*256 of K runs as fp8(e4m3) DoubleRow matmuls (256-contraction per
instruction, 2x PE throughput), the remaining K in fp16. Operands are
pre-scaled by powers of two (x*2^5, w*2^10) so both parts accumulate in one
fp32 PSUM group; drains rescale by 2^-15 and add the bias in one fused op.

Schedule: diagonally-staggered warmup over 4 o-groups (k-major, consuming the
x stream as it arrives), then o-major steady phase with chain boundaries
software-pipelined (the next o-group's first matmul issues before the current
group's last, hiding the chain-start LDWEIGHTS hiccup).
"""

import numpy as np
import ml_dtypes
import concourse.bacc as bacc
import concourse.mybir as mybir
import concourse.tile as tile
from concourse.bass_utils import run_bass_kernel_spmd

# problem shapes (hardcoded per contract)
B, S = 2, 2048
IN_FEATURES = 4096
OUT_FEATURES = 4096
NTOKENS = B * S                  # 4096

NCORES = 8
TG, OG = 4, 2                    # token groups x out-feature groups
T = NTOKENS // TG                # 1024 tokens per core
O = OUT_FEATURES // OG           # 2048 out features per core
P = 128
KT = IN_FEATURES // P            # 32 contraction tiles
OT = O // P                      # 16 out-feature tiles per core
NTOK = 512                       # moving free dim per matmul
TB = T // NTOK                   # 2 token blocks per core

N8 = 7                           # fp8 DoubleRow pairs (256 K each)
K8 = N8 * 2 * P                  # K prefix contracted in fp8
KB = KT - N8 * 2                 # fp16 k-tiles

SX = 32.0                        # x pre-scale (power of 2)
SW = 1024.0                      # w pre-scale (power of 2)
SCALE_OUT = 1.0 / (SX * SW)      # exact 2^-15

F32 = mybir.dt.float32
HP = mybir.dt.float16            # dtype of the non-fp8 contraction part
FP8 = mybir.dt.float8e4
DR = mybir.MatmulPerfMode.DoubleRow
NP_FP8 = ml_dtypes.float8_e4m3   # TRN FP8_EXP4: max normal 240
NP_HP = np.float16

# knobs for test.py
TRACE = False
LAST_RESULTS = None

# contraction units per (o, tb) psum chain: N8 DoubleRow + KB fp16
UNITS = [("dr", i) for i in range(N8)] + [("bf", i) for i in range(KB)]
NU = len(UNITS)                  # 26
# warmup consumption order: DR units interleaved into the fp16 stream so the
# later x8 slabs (which arrive behind wb on the scalar queue) are needed later
WUNITS = [0, 1, 7, 8, 2, 9, 10, 3, 11, 12, 4, 13, 14, 5, 15, 16, 6] + list(
    range(17, 25)
)
assert sorted(WUNITS) == list(range(NU))
LASTW = 4                        # trailing units drained o-major in warmup
NWARM = 4                        # o-groups processed k-major during warmup
STAG = 2                         # warmup diagonal stagger (units per o-group)


def build_nc():
    nc = bacc.Bacc()
    x8 = nc.dram_tensor("x8", [N8, P, 2, T], FP8, kind="ExternalInput")
    xb = nc.dram_tensor("xb", [KB, P, T], HP, kind="ExternalInput")
    w8 = nc.dram_tensor("w8", [OT, P, N8, 2, P], FP8, kind="ExternalInput")
    wb = nc.dram_tensor("wb", [OT, P, KB, P], HP, kind="ExternalInput")
    bo = nc.dram_tensor("bo", [P, OT], F32, kind="ExternalInput")
    out = nc.dram_tensor("out", [OT, TB, P, NTOK], F32, kind="ExternalOutput")

    with tile.TileContext(nc) as tc:
        with (
            tc.tile_pool(name="x_sb", bufs=1) as x_sb,
            tc.tile_pool(name="w8_sb", bufs=7) as w8_sb,
            tc.tile_pool(name="wb_sb", bufs=6) as wb_sb,
            tc.tile_pool(name="o_sb", bufs=4) as o_sb,
            tc.tile_pool(name="ps", bufs=8, space="PSUM") as ps,
        ):
            bo_t = x_sb.tile([P, OT], F32, tag="bo")

            # PE HAM warmup: dummy matmuls on memset data fill the dead time
            # while the first DMAs land, so real matmuls start at 2.4 GHz
            dummy_sb = x_sb.tile([P, NTOK], HP, tag="dummy")
            nc.vector.memset(dummy_sb.bitcast(F32), 0.0)
            ps_d = ps.tile([P, NTOK], F32, tag="ps", name="ps_dummy")
            for _ in range(10):
                nc.tensor.matmul(
                    ps_d, dummy_sb[:, :P], dummy_sb, start=True, stop=True
                )

            w8t, wbt, x8_t, xb_t = {}, {}, {}, {}

            def load_w8(o):
                t = w8_sb.tile([P, N8, 2, P], FP8, tag="w8t", name=f"w8_{o}")
                nc.sync.dma_start(out=t, in_=w8[o])
                w8t[o] = t

            def load_wb(o, eng):
                t = wb_sb.tile([P, KB, P], HP, tag="wbt", name=f"wb_{o}")
                eng.dma_start(out=t, in_=wb[o])
                wbt[o] = t

            def load_x8(i, eng):
                t = x_sb.tile([P, 2, T], FP8, tag=f"x8_{i}")
                eng.dma_start(out=t, in_=x8[i])
                x8_t[i] = t

            def load_xb(i, eng):
                t = x_sb.tile([P, T], HP, tag=f"xb_{i}")
                eng.dma_start(out=t, in_=xb[i])
                xb_t[i] = t

            def unit_mm(psum, o, u, tb, start, stop):
                kind, i = UNITS[u]
                if kind == "dr":
                    nc.tensor.matmul(
                        psum,
                        w8t[o][:, i],
                        x8_t[i][:, :, tb * NTOK : (tb + 1) * NTOK],
                        start=start,
                        stop=stop,
                        perf_mode=DR,
                    )
                else:
                    nc.tensor.matmul(
                        psum,
                        wbt[o][:, i],
                        xb_t[i][:, tb * NTOK : (tb + 1) * NTOK],
                        start=start,
                        stop=stop,
                    )

            o_pair = {}

            def drain(o, tb, psum):
                if tb == 0:
                    o_pair[o] = o_sb.tile([P, TB, NTOK], F32, tag="ot", name=f"ot_{o}")
                o_t = o_pair[o][:, tb]
                # psum * 2^-15 + bias in one fused op; alternate engines so
                # consecutive drains run in parallel
                if (o * TB + tb) % 2 == 0:
                    nc.scalar.activation(
                        o_t,
                        psum,
                        mybir.ActivationFunctionType.Identity,
                        bias=bo_t[:, o : o + 1],
                        scale=SCALE_OUT,
                    )
                else:
                    nc.vector.tensor_scalar(
                        o_t,
                        psum,
                        SCALE_OUT,
                        bo_t[:, o : o + 1],
                        mybir.AluOpType.mult,
                        mybir.AluOpType.add,
                    )
                if tb == TB - 1:
                    nc.scalar.dma_start(
                        out=out[o].rearrange("two p n -> p two n"), in_=o_pair[o]
                    )

            # ---- DMA issue for the warmup span ----
            # x8 + w8 go on the two hardware-DGE queues (sync/scalar) in
            # consumption order -- gpsimd's software DGE adds latency, so it
            # only carries the latency-tolerant xb bulk stream
            load_x8(0, nc.sync)
            load_w8(0)
            load_x8(1, nc.sync)
            for o in range(1, NWARM):
                load_w8(o)
            nc.sync.dma_start(out=bo_t, in_=bo[:, :])
            load_wb(0, nc.scalar)
            load_wb(1, nc.scalar)
            load_x8(2, nc.scalar)
            load_x8(3, nc.scalar)
            load_wb(2, nc.scalar)
            load_x8(4, nc.scalar)
            load_wb(3, nc.scalar)
            load_x8(5, nc.scalar)
            load_x8(6, nc.scalar)
            for i in range(13):
                load_xb(i, nc.gpsimd)
            for i in range(13, KB):
                load_xb(i, nc.scalar)

            # ---- warmup: diagonally staggered k-major over NWARM o-groups ----
            # o-group o starts STAG units late so its weights needn't arrive
            # all at once with o=0's
            psums = {
                (o, tb): ps.tile([P, NTOK], F32, tag="ps", name=f"psw_{o}_{tb}")
                for o in range(NWARM)
                for tb in range(TB)
            }
            NWU = NU - LASTW
            for s in range(NWU + STAG * (NWARM - 1)):
                for o in range(NWARM):
                    idx = s - STAG * o
                    if 0 <= idx < NWU:
                        for tb in range(TB):
                            unit_mm(psums[o, tb], o, WUNITS[idx], tb, idx == 0, False)

            # ---- unified tail: warmup chain closings + steady o-groups, with
            # chain boundaries software-pipelined (next group's first unit
            # issues before this group's last unit + drains)
            pend = None
            for o in range(OT):
                if o < NWARM:
                    psg = {tb: psums[o, tb] for tb in range(TB)}
                    order = WUNITS[NU - LASTW :]
                    opens = False
                else:
                    load_w8(o)
                    load_wb(o, nc.sync if o % 2 == 0 else nc.scalar)
                    psg = {
                        tb: ps.tile([P, NTOK], F32, tag="ps", name=f"ps_{o}_{tb}")
                        for tb in range(TB)
                    }
                    # alternate chain direction so fp8 LDWEIGHTS bursts don't
                    # pile up at every boundary
                    order = list(range(NU)) if o % 2 == 0 else list(range(NU))[::-1]
                    opens = True
                for tb in range(TB):
                    unit_mm(psg[tb], o, order[0], tb, opens, False)
                if pend is not None:
                    po, pps, plast = pend
                    for tb in range(TB):
                        unit_mm(pps[tb], po, plast, tb, False, True)
                    for tb in range(TB):
                        drain(po, tb, pps[tb])
                for u in order[1:-1]:
                    for tb in range(TB):
                        unit_mm(psg[tb], o, u, tb, False, False)
                pend = (o, psg, order[-1])
            po, pps, plast = pend
            for tb in range(TB):
                unit_mm(pps[tb], po, plast, tb, False, True)
            for tb in range(TB):
                drain(po, tb, pps[tb])
    nc.finalize()
    return nc


_NC = None


def _get_nc():
    global _NC
    if _NC is None:
        _NC = build_nc()
    return _NC


def _build_wfull(weights, input_indices, output_indices):
    """Wfull[k, o] = sum over blocks/dups of weights[n, j, i]."""
    ii = np.asarray(input_indices).astype(np.int64)     # [NBLK, BI]
    oi = np.asarray(output_indices).astype(np.int64)    # [NBLK, BO]
    w = np.asarray(weights, dtype=np.float64)           # [NBLK, BO, BI]
    flat = (ii[:, :, None] * OUT_FEATURES + oi[:, None, :]).ravel()  # [n, i, j]
    vals = np.ascontiguousarray(np.swapaxes(w, 1, 2)).ravel()        # [n, i, j]
    wfull = np.bincount(flat, weights=vals, minlength=IN_FEATURES * OUT_FEATURES)
    return wfull.reshape(IN_FEATURES, OUT_FEATURES)


def _to_fp8(a):
    return np.clip(a, -240.0, 240.0).astype(NP_FP8)


def prepare_in_maps(x, weights, bias, input_indices, output_indices):
    x = np.asarray(x, dtype=np.float32).reshape(NTOKENS, IN_FEATURES)
    bias = np.asarray(bias, dtype=np.float32)
    wfull = _build_wfull(weights, input_indices, output_indices)

    # permute k-tiles so the 2*N8 with the least quantization-error energy
    # (sum_k ||x[:,k]||^2 * ||w[k,:]||^2) are the ones computed in fp8
    xsq = (x.astype(np.float64) ** 2).sum(0)
    wsq = (wfull ** 2).sum(1)
    mass = (xsq * wsq).reshape(KT, P).sum(1)
    order = np.argsort(mass)
    perm = np.concatenate([np.sort(order[: 2 * N8]), np.sort(order[2 * N8 :])])
    kperm = (perm[:, None] * P + np.arange(P)).ravel()
    x = x[:, kperm]
    wfull = wfull[kperm, :]

    # quantize once globally (scales are powers of two; folded out in drain)
    x8_full = _to_fp8(x[:, :K8].astype(np.float64) * SX)         # [NT, K8]
    xb_full = (x[:, K8:] * np.float32(SX)).astype(NP_HP)         # [NT, K-K8]
    w8_full = _to_fp8(wfull[:K8, :] * SW)                        # [K8, OF]
    wb_full = (wfull[K8:, :] * SW).astype(NP_HP)                 # [K-K8, OF]

    in_maps = []
    for c in range(NCORES):
        tg, og = divmod(c, OG)
        tsl = slice(tg * T, (tg + 1) * T)
        osl = slice(og * O, (og + 1) * O)
        # x8: [N8, P, 2, T] ; k = (2*kk + j)*128 + p
        x8c = np.ascontiguousarray(
            x8_full[tsl].T.reshape(N8, 2, P, T).transpose(0, 2, 1, 3)
        )
        # xb: [KB, P, T]
        xbc = np.ascontiguousarray(xb_full[tsl].T.reshape(KB, P, T))
        # w8: [OT, P, N8, 2, P]
        w8c = np.ascontiguousarray(
            w8_full[:, osl].reshape(N8, 2, P, OT, P).transpose(3, 2, 0, 1, 4)
        )
        # wb: [OT, P, KB, P]
        wbc = np.ascontiguousarray(
            wb_full[:, osl].reshape(KB, P, OT, P).transpose(2, 1, 0, 3)
        )
        boc = np.ascontiguousarray(bias[osl].reshape(OT, P).T)
        in_maps.append({"x8": x8c, "xb": xbc, "w8": w8c, "wb": wbc, "bo": boc})
    return in_maps


def assemble_output(core_outs):
    full = np.empty((NTOKENS, OUT_FEATURES), np.float32)
    for c in range(NCORES):
        tg, og = divmod(c, OG)
        o4 = np.asarray(core_outs[c])                    # [OT, TB, P, NTOK]
        blk = o4.transpose(1, 3, 0, 2).reshape(T, O)     # [t, o]
        full[tg * T : (tg + 1) * T, og * O : (og + 1) * O] = blk
    return full.reshape(B, S, OUT_FEATURES)


def kernel(x, weights, bias, input_indices, output_indices):
    global LAST_RESULTS
    in_maps = prepare_in_maps(x, weights, bias, input_indices, output_indices)
    nc = _get_nc()
    res = run_bass_kernel_spmd(nc, in_maps, list(range(NCORES)))
    LAST_RESULTS = res
    return assemble_output([res.results[c]["out"] for c in range(NCORES)])


# revision 20
# speedup vs baseline: 1.1982x; 1.0426x over previous
"""Trainium2 kernel for nn_BlockLinear: gather -> per-block GEMM -> scatter-add.

The whole op is linear in x, so gather/einsum/scatter fold into one dense GEMM
out[t, o] = sum_k x[t, k] * Wfull[k, o] + bias[o], with Wfull built on host
(bincount scatter-add, exact fp64 accumulation). The GEMM runs on 8
NeuronCores, sharded 2D: 4 token groups x 2 out-feature groups.

Mixed-precision contraction split (rel-err budget 2e-2, measured 1.8e-2):
the first N8*256 of K runs as fp8(e4m3) DoubleRow matmuls (256-contraction per
instruction, 2x PE throughput), the remaining K in fp16. Operands are
pre-scaled by powers of two (x*2^5, w*2^10) so both parts accumulate in one
fp32 PSUM group; drains rescale by 2^-15 and add the bias in one fused op.

Schedule: diagonally-staggered warmup over 4 o-groups (k-major, consuming the
x stream as it arrives), then o-major steady phase with chain boundaries
software-pipelined (the next o-group's first matmul issues before the current
group's last, hiding the chain-start LDWEIGHTS hiccup).
"""

import numpy as np
import ml_dtypes
import concourse.bacc as bacc
import concourse.mybir as mybir
import concourse.tile as tile
from concourse.bass_utils import run_bass_kernel_spmd

# problem shapes (hardcoded per contract)
B, S = 2, 2048
IN_FEATURES = 4096
OUT_FEATURES = 4096
NTOKENS = B * S                  # 4096

NCORES = 8
TG, OG = 4, 2                    # token groups x out-feature groups
T = NTOKENS // TG                # 1024 tokens per core
O = OUT_FEATURES // OG           # 2048 out features per core
P = 128
KT = IN_FEATURES // P            # 32 contraction tiles
OT = O // P                      # 16 out-feature tiles per core
NTOK = 512                       # moving free dim per matmul
TB = T // NTOK                   # 2 token blocks per core

N8 = 7                           # fp8 DoubleRow pairs (256 K each)
K8 = N8 * 2 * P                  # K prefix contracted in fp8
KB = KT - N8 * 2                 # fp16 k-tiles

SX = 32.0                        # x pre-scale (power of 2)
SW = 1024.0                      # w pre-scale (power of 2)
SCALE_OUT = 1.0 / (SX * SW)      # exact 2^-15

F32 = mybir.dt.float32
HP = mybir.dt.float16            # dtype of the non-fp8 contraction part
FP8 = mybir.dt.float8e4
DR = mybir.MatmulPerfMode.DoubleRow
NP_FP8 = ml_dtypes.float8_e4m3   # TRN FP8_EXP4: max normal 240
NP_HP = np.float16

# knobs for test.py
TRACE = False
LAST_RESULTS = None

# contraction units per (o, tb) psum chain: N8 DoubleRow + KB fp16
UNITS = [("dr", i) for i in range(N8)] + [("bf", i) for i in range(KB)]
NU = len(UNITS)                  # 26
LASTW = 4                        # trailing units drained o-major in warmup
NWARM = 4                        # o-groups processed k-major during warmup
STAG = 2                         # warmup diagonal stagger (units per o-group)


def build_nc():
    nc = bacc.Bacc()
    x8 = nc.dram_tensor("x8", [N8, P, 2, T], FP8, kind="ExternalInput")
    xb = nc.dram_tensor("xb", [KB, P, T], HP, kind="ExternalInput")
    w8 = nc.dram_tensor("w8", [OT, P, N8, 2, P], FP8, kind="ExternalInput")
    wb = nc.dram_tensor("wb", [OT, P, KB, P], HP, kind="ExternalInput")
    bo = nc.dram_tensor("bo", [P, OT], F32, kind="ExternalInput")
    out = nc.dram_tensor("out", [OT, TB, P, NTOK], F32, kind="ExternalOutput")

    with tile.TileContext(nc) as tc:
        with (
            tc.tile_pool(name="x_sb", bufs=1) as x_sb,
            tc.tile_pool(name="w8_sb", bufs=5) as w8_sb,
            tc.tile_pool(name="wb_sb", bufs=5) as wb_sb,
            tc.tile_pool(name="o_sb", bufs=4) as o_sb,
            tc.tile_pool(name="ps", bufs=8, space="PSUM") as ps,
        ):
            bo_t = x_sb.tile([P, OT], F32, tag="bo")

            # PE HAM warmup: dummy matmuls on memset data fill the dead time
            # while the first DMAs land, so real matmuls start at 2.4 GHz
            dummy_sb = x_sb.tile([P, NTOK], HP, tag="dummy")
            nc.vector.memset(dummy_sb.bitcast(F32), 0.0)
            ps_d = ps.tile([P, NTOK], F32, tag="ps", name="ps_dummy")
            for _ in range(10):
                nc.tensor.matmul(
                    ps_d, dummy_sb[:, :P], dummy_sb, start=True, stop=True
                )

            w8t, wbt, x8_t, xb_t = {}, {}, {}, {}

            def load_w8(o):
                t = w8_sb.tile([P, N8, 2, P], FP8, tag="w8t", name=f"w8_{o}")
                nc.sync.dma_start(out=t, in_=w8[o])
                w8t[o] = t

            def load_wb(o, eng):
                t = wb_sb.tile([P, KB, P], HP, tag="wbt", name=f"wb_{o}")
                eng.dma_start(out=t, in_=wb[o])
                wbt[o] = t

            def load_x8(i, eng):
                t = x_sb.tile([P, 2, T], FP8, tag=f"x8_{i}")
                eng.dma_start(out=t, in_=x8[i])
                x8_t[i] = t

            def load_xb(i, eng):
                t = x_sb.tile([P, T], HP, tag=f"xb_{i}")
                eng.dma_start(out=t, in_=xb[i])
                xb_t[i] = t

            def unit_mm(psum, o, u, tb, start, stop):
                kind, i = UNITS[u]
                if kind == "dr":
                    nc.tensor.matmul(
                        psum,
                        w8t[o][:, i],
                        x8_t[i][:, :, tb * NTOK : (tb + 1) * NTOK],
                        start=start,
                        stop=stop,
                        perf_mode=DR,
                    )
                else:
                    nc.tensor.matmul(
                        psum,
                        wbt[o][:, i],
                        xb_t[i][:, tb * NTOK : (tb + 1) * NTOK],
                        start=start,
                        stop=stop,
                    )

            o_pair = {}

            def drain(o, tb, psum):
                if tb == 0:
                    o_pair[o] = o_sb.tile([P, TB, NTOK], F32, tag="ot", name=f"ot_{o}")
                o_t = o_pair[o][:, tb]
                # psum * 2^-15 + bias in one fused op; alternate engines so
                # consecutive drains run in parallel
                if (o * TB + tb) % 2 == 0:
                    nc.scalar.activation(
                        o_t,
                        psum,
                        mybir.ActivationFunctionType.Identity,
                        bias=bo_t[:, o : o + 1],
                        scale=SCALE_OUT,
                    )
                else:
                    nc.vector.tensor_scalar(
                        o_t,
                        psum,
                        SCALE_OUT,
                        bo_t[:, o : o + 1],
                        mybir.AluOpType.mult,
                        mybir.AluOpType.add,
                    )
                if tb == TB - 1:
                    nc.scalar.dma_start(
                        out=out[o].rearrange("two p n -> p two n"), in_=o_pair[o]
                    )

            # ---- DMA issue for the warmup span ----
            # x8 + w8 go on the two hardware-DGE queues (sync/scalar) in
            # consumption order -- gpsimd's software DGE adds latency, so it
            # only carries the latency-tolerant xb bulk stream
            load_x8(0, nc.sync)
            load_w8(0)
            load_x8(1, nc.sync)
            for i in range(2, N8):
                load_x8(i, nc.scalar)
            for o in range(1, NWARM):
                load_w8(o)
            nc.sync.dma_start(out=bo_t, in_=bo[:, :])
            for i in range(13):
                load_xb(i, nc.gpsimd)
            for o in range(NWARM):
                load_wb(o, nc.scalar)
            for i in range(13, KB):
                load_xb(i, nc.scalar)

            # ---- warmup: diagonally staggered k-major over NWARM o-groups ----
            # o-group o starts STAG units late so its weights needn't arrive
            # all at once with o=0's
            psums = {
                (o, tb): ps.tile([P, NTOK], F32, tag="ps", name=f"psw_{o}_{tb}")
                for o in range(NWARM)
                for tb in range(TB)
            }
            NWU = NU - LASTW
            for s in range(NWU + STAG * (NWARM - 1)):
                for o in range(NWARM):
                    u = s - STAG * o
                    if 0 <= u < NWU:
                        for tb in range(TB):
                            unit_mm(psums[o, tb], o, u, tb, u == 0, False)

            # ---- unified tail: warmup chain closings + steady o-groups, with
            # chain boundaries software-pipelined (next group's first unit
            # issues before this group's last unit + drains)
            pend = None
            for o in range(OT):
                if o < NWARM:
                    psg = {tb: psums[o, tb] for tb in range(TB)}
                    order = list(range(NU - LASTW, NU))
                    opens = False
                else:
                    load_w8(o)
                    load_wb(o, nc.sync if o % 2 == 0 else nc.scalar)
                    psg = {
                        tb: ps.tile([P, NTOK], F32, tag="ps", name=f"ps_{o}_{tb}")
                        for tb in range(TB)
                    }
                    # alternate chain direction so fp8 LDWEIGHTS bursts don't
                    # pile up at every boundary
                    order = list(range(NU)) if o % 2 == 0 else list(range(NU))[::-1]
                    opens = True
                for tb in range(TB):
                    unit_mm(psg[tb], o, order[0], tb, opens, False)
                if pend is not None:
                    po, pps, plast = pend
                    for tb in range(TB):
                        unit_mm(pps[tb], po, plast, tb, False, True)
                    for tb in range(TB):
                        drain(po, tb, pps[tb])
                for u in order[1:-1]:
                    for tb in range(TB):
                        unit_mm(psg[tb], o, u, tb, False, False)
                pend = (o, psg, order[-1])
            po, pps, plast = pend
            for tb in range(TB):
                unit_mm(pps[tb], po, plast, tb, False, True)
            for tb in range(TB):
                drain(po, tb, pps[tb])
    nc.finalize()
    return nc


_NC = None


def _get_nc():
    global _NC
    if _NC is None:
        _NC = build_nc()
    return _NC


def _build_wfull(weights, input_indices, output_indices):
    """Wfull[k, o] = sum over blocks/dups of weights[n, j, i]."""
    ii = np.asarray(input_indices).astype(np.int64)     # [NBLK, BI]
    oi = np.asarray(output_indices).astype(np.int64)    # [NBLK, BO]
    w = np.asarray(weights, dtype=np.float64)           # [NBLK, BO, BI]
    flat = (ii[:, :, None] * OUT_FEATURES + oi[:, None, :]).ravel()  # [n, i, j]
    vals = np.ascontiguousarray(np.swapaxes(w, 1, 2)).ravel()        # [n, i, j]
    wfull = np.bincount(flat, weights=vals, minlength=IN_FEATURES * OUT_FEATURES)
    return wfull.reshape(IN_FEATURES, OUT_FEATURES)


def _to_fp8(a):
    return np.clip(a, -240.0, 240.0).astype(NP_FP8)


def prepare_in_maps(x, weights, bias, input_indices, output_indices):
    x = np.asarray(x, dtype=np.float32).reshape(NTOKENS, IN_FEATURES)
    bias = np.asarray(bias, dtype=np.float32)
    wfull = _build_wfull(weights, input_indices, output_indices)

    # permute k-tiles so the 2*N8 with the least quantization-error energy
    # (sum_k ||x[:,k]||^2 * ||w[k,:]||^2) are the ones computed in fp8
    xsq = (x.astype(np.float64) ** 2).sum(0)
    wsq = (wfull ** 2).sum(1)
    mass = (xsq * wsq).reshape(KT, P).sum(1)
    order = np.argsort(mass)
    perm = np.concatenate([np.sort(order[: 2 * N8]), np.sort(order[2 * N8 :])])
    kperm = (perm[:, None] * P + np.arange(P)).ravel()
    x = x[:, kperm]
    wfull = wfull[kperm, :]

    # quantize once globally (scales are powers of two; folded out in drain)
    x8_full = _to_fp8(x[:, :K8].astype(np.float64) * SX)         # [NT, K8]
    xb_full = (x[:, K8:] * np.float32(SX)).astype(NP_HP)         # [NT, K-K8]
    w8_full = _to_fp8(wfull[:K8, :] * SW)                        # [K8, OF]
    wb_full = (wfull[K8:, :] * SW).astype(NP_HP)                 # [K-K8, OF]

    in_maps = []
    for c in range(NCORES):
        tg, og = divmod(c, OG)
        tsl = slice(tg * T, (tg + 1) * T)
        osl = slice(og * O, (og + 1) * O)
        # x8: [N8, P, 2, T] ; k = (2*kk + j)*128 + p
        x8c = np.ascontiguousarray(
            x8_full[tsl].T.reshape(N8, 2, P, T).transpose(0, 2, 1, 3)
        )
        # xb: [KB, P, T]
        xbc = np.ascontiguousarray(xb_full[tsl].T.reshape(KB, P, T))
        # w8: [OT, P, N8, 2, P]
        w8c = np.ascontiguousarray(
            w8_full[:, osl].reshape(N8, 2, P, OT, P).transpose(3, 2, 0, 1, 4)
        )
        # wb: [OT, P, KB, P]
        wbc = np.ascontiguousarray(
            wb_full[:, osl].reshape(KB, P, OT, P).transpose(2, 1, 0, 3)
        )
        boc = np.ascontiguousarray(bias[osl].reshape(OT, P).T)
        in_maps.append({"x8": x8c, "xb": xbc, "w8": w8c, "wb": wbc, "bo": boc})
    return in_maps


def assemble_output(core_outs):
    full = np.empty((NTOKENS, OUT_FEATURES), np.float32)
    for c in range(NCORES):
        tg, og = divmod(c, OG)
        o4 = np.asarray(core_outs[c])                    # [OT, TB, P, NTOK]
        blk = o4.transpose(1, 3, 0, 2).reshape(T, O)     # [t, o]
        full[tg * T : (tg + 1) * T, og * O : (og + 1) * O] = blk
    return full.reshape(B, S, OUT_FEATURES)


def kernel(x, weights, bias, input_indices, output_indices):
    global LAST_RESULTS
    in_maps = prepare_in_maps(x, weights, bias, input_indices, output_indices)
    nc = _get_nc()
    res = run_bass_kernel_spmd(nc, in_maps, list(range(NCORES)))
    LAST_RESULTS = res
    return assemble_output([res.results[c]["out"] for c in range(NCORES)])
